# revision 1
# baseline (speedup 1.0000x reference)
"""Trainium2 Bass kernel for CapsNet DigitCaps dynamic routing (nn_DigitCaps).

Reference computation:
    u_hat[b,r,j,o] = W[r,j,o,:] @ x[b,r,:]        B,R,J,O,I = 512,1152,10,16,8
    b_ij = 0; 3 routing iterations:
        c = softmax(b_ij, axis=0)                  # over routes r, per j
        s[b,j,o] = sum_r c[r,j] * u_hat[b,r,j,o]
        v = squash(s) = s*|s|/(1+s^2)              # elementwise
        b_ij += mean_b sum_o u_hat[b,r,j,o]*v[b,j,o]
    return v[..., None]

Kernel strategy (data-parallel over batch, 8 cores, 64 rows each; u_hat is
never materialized):
    s    = X @ (c ⊙ W2)          X: [64, 9216]=[b,(r,i)], W2: [9216,160]=[(r,i),(j,o)]
    M    = X^T @ V               [9216, 160] per-core partial (batch outer product)
    bupd = sum_{i,o} W2 ⊙ M      computed as DVE product + PE block-ones matmuls
    bupd is AllReduce-summed across cores ([1152,10] = 46KB), softmax + weight
    scaling are computed redundantly on every core.
Matmuls run as float32r (~1 cyc/row at N>=256, rel err ~1.5e-4); everything
else is fp32.
"""
import os
import numpy as np
from contextlib import ExitStack

import concourse.bacc as bacc
import concourse.bass as bass
import concourse.tile as tile
from concourse import mybir
from concourse.bass_utils import run_bass_kernel_spmd

F32 = mybir.dt.float32
F32R = mybir.dt.float32r
BF16 = mybir.dt.bfloat16

B, R, J, O, I = 512, 1152, 10, 16, 8
N_CORES = 8
BL = B // N_CORES          # 64 batch rows per core
RI = R * I                 # 9216
NJO = J * O                # 160
NPAD = 256                 # padded matmul free dim (fp32r fast path needs >=256)
KT = RI // 128             # 72 contraction chunks
NUM_ITER = 3


def emit_algorithm(nc, tc, ctx, tensors, pools, out_d, flags=()):
    """Emit one full 3-iteration routing computation."""
    (xT_s, x_s, W2_s, I8S_s, REPL_s, ONES_s, RONES_s) = tensors
    (sp, vq, wc_pool, p_pool, dram_pool,
     ps_s, ps_m, ps_bup, ps_crep, ps_small) = pools
    ps_tiny = ps_small

    if "only_ar" in flags or "only_ag" in flags:
        prev = sp.tile([128, 90], F32, tag="b_upd")
        nc.vector.tensor_copy(prev[:], W2_s[:, 0:90].bitcast(F32))
        for a in range(2):
            nxt = sp.tile([128, 90], F32, tag=f"bst{a}")
            if "only_ar" in flags:
                cc_in = dram_pool.tile([128, 90], F32, tag="cc_in")
                cc_out = dram_pool.tile([128, 90], F32, tag="cc_out")
                nc.sync.dma_start(cc_in[:], prev[:])
                nc.gpsimd.collective_compute(
                    "AllReduce", mybir.AluOpType.add,
                    replica_groups=[list(range(N_CORES))],
                    ins=[cc_in.opt()], outs=[cc_out.opt()])
                nc.sync.dma_start(nxt[:], cc_out[:])
            else:
                cc_in = dram_pool.tile([128, 90], F32, tag="cc_in")
                cc_out = dram_pool.tile([N_CORES * 128, 90], F32, tag="cc_outg")
                nc.sync.dma_start(cc_in[:], prev[:])
                nc.gpsimd.collective_compute(
                    "AllGather", mybir.AluOpType.bypass,
                    replica_groups=[list(range(N_CORES))],
                    ins=[cc_in.opt()], outs=[cc_out.opt()])
                gath = sp.tile([128, 8 * 90], F32, tag="gath")
                nc.sync.dma_start(
                    gath[:].rearrange("p (k f) -> p k f", f=90),
                    cc_out[:].rearrange("(k p) f -> p k f", p=128))
                nc.vector.tensor_reduce(
                    nxt[:], gath[:].rearrange("p (k f) -> p f k", f=90),
                    axis=mybir.AxisListType.X, op=mybir.AluOpType.add)
            prev = nxt
        vout = vq.tile([BL, NJO], F32, tag="vout")
        nc.vector.tensor_copy(vout[0:64, 0:90], prev[0:64, :])
        nc.vector.tensor_copy(vout[0:64, 90:160], prev[0:64, 0:70])
        nc.sync.dma_start(out_d[:], vout[:])
        return

    b_state = None
    for it in range(NUM_ITER):
        # ---- c preparation (softmax over r, replicated to (r,i) chunks) ----
        crep = None
        zinv_b = None
        if it > 0 and "skip_sm_chain" in flags:
            # keep the crep replication matmuls, skip the softmax scalar chain
            cb = sp.tile([128, 90], F32, tag="cb")
            nc.vector.memset(cb[:], 0.001)
            crep = sp.tile([128, 720], F32, tag="crep")
            for half, (m0, nm) in enumerate([(0, 5), (5, 4)]):
                cps = ps_crep.tile([128, 80 * nm], F32, tag="sc")
                for g in range(8):
                    nc.tensor.matmul(
                        cps[:].rearrange("p (m x) -> p m x", x=80)[:, :, g * 10:g * 10 + 10],
                        REPL_s[:, g * 128:(g + 1) * 128],
                        cb[:, m0 * 10:(m0 + nm) * 10].rearrange("p (m j) -> p m j", j=J),
                        start=True, stop=True)
                nc.scalar.activation(crep[:, m0 * 80:(m0 + nm) * 80], cps[:],
                                     mybir.ActivationFunctionType.Copy)
        elif it > 0 and "skip_softmax" not in flags:
            # Deferred softmax normalization: use unnormalized e = exp(b) for
            # the weight scaling; the per-j 1/sum(e) factor is applied to the
            # matmul OUTPUT during squash (it commutes through the matmul).
            # b_state: [128, 90] layout [p, (rb, j)] with r = rb*128 + p
            e = sp.tile([128, 90], F32, tag="e")
            nc.scalar.activation(e[:], b_state[:], mybir.ActivationFunctionType.Exp)
            # replicate e[r, j] -> crep, g-major layout: col = g*90 + m*10 + j
            # (one full-width N=90 matmul per partition-group g)
            crep = sp.tile([128, 720], F32, tag="crep")
            for half in range(2):
                cps = ps_crep.tile([128, 360], F32, tag="sc")
                for gg in range(4):
                    g = half * 4 + gg
                    nc.tensor.matmul(
                        cps[:, gg * 90:(gg + 1) * 90],
                        REPL_s[:, g * 128:(g + 1) * 128],
                        e[:], start=True, stop=True)
                nc.vector.tensor_copy(crep[:, half * 360:(half + 1) * 360], cps[:])
            # Z_j = sum_r e[r, j]; zinv_b = 1/Z broadcast over the 64 b-rows.
            # Runs concurrently with the s-matmul below (only squash needs it).
            sums_ps = ps_tiny.tile([1, 90], F32, tag="tiny")
            nc.tensor.matmul(
                sums_ps[:].rearrange("p (j rb) -> p j rb", rb=9),
                ONES_s[:, 0:1],
                e[:].rearrange("p (rb j) -> p j rb", j=J),
                start=True, stop=True)
            zsum = sp.tile([1, 10], F32, tag="zsum")
            nc.vector.tensor_reduce(
                zsum[:], sums_ps[:].rearrange("p (j rb) -> p j rb", rb=9),
                axis=mybir.AxisListType.X, op=mybir.AluOpType.add)
            zinv = sp.tile([1, 10], F32, tag="zinv")
            nc.vector.reciprocal(zinv[:], zsum[:])
            zb_ps = ps_tiny.tile([BL, 10], F32, tag="tiny")
            nc.tensor.matmul(zb_ps[:], RONES_s[:, 0:BL], zinv[:],
                             start=True, stop=True)
            zinv_b = sp.tile([BL, 10], F32, tag="zinv_b")
            nc.vector.tensor_copy(zinv_b[:], zb_ps[:])

        if it > 0 and "junk_crep" in flags and crep is None:
            crep = sp.tile([128, 720], F32, tag="crep")
            nc.vector.memset(crep[:], 0.001)

        # ---- s matmul (+ weight scaling, 4-chunk fused) ----
        s_ps = ps_s.tile([BL, NPAD], F32, tag="sc")
        use_scale = not (it == 0 or crep is None or "skip_scale" in flags)
        FS = 8  # chunks per fused scale op
        for k4 in range(KT // FS):
            if use_scale:
                wc = wc_pool.tile([128, FS * NPAD], F32R, tag="wc")
                in0 = W2_s[:, k4 * FS * NPAD:(k4 + 1) * FS * NPAD].bitcast(F32) \
                    .rearrange("p (c n) -> p c n", n=NPAD)[:, :, 0:NJO] \
                    .rearrange("p c (j o) -> p c j o", o=O)
                in1 = crep[:].rearrange("p (g m j) -> p g m j", m=9, j=J) \
                    [:, :, k4, :].rearrange("p g (j o) -> p g j o", o=1)
                i0b, i1b = bass.broadcast_tensor_aps(in0, in1)
                nc.vector.tensor_tensor(
                    wc[:].rearrange("p (c n) -> p c n", n=NPAD)[:, :, 0:NJO]
                    .rearrange("p c (j o) -> p c j o", o=O),
                    i0b, i1b, op=mybir.AluOpType.mult)
            for dk in range(FS):
                k = k4 * FS + dk
                if use_scale:
                    rhs = wc[:, dk * NPAD:(dk + 1) * NPAD]
                else:
                    rhs = W2_s[:, k * NPAD:(k + 1) * NPAD]
                nc.tensor.matmul(s_ps[:], xT_s[:, k * BL:(k + 1) * BL], rhs,
                                 start=(k == 0), stop=(k == KT - 1))

        # ---- squash (with deferred softmax normalization for it > 0) ----
        if use_scale and zinv_b is not None:
            s_n = sp.tile([BL, NJO], F32, tag="s_n")
            i0 = s_ps[:, 0:NJO].rearrange("p (j o) -> p j o", o=O)
            i1 = zinv_b[:].rearrange("p (j o) -> p j o", o=1)
            i0b, i1b = bass.broadcast_tensor_aps(i0, i1)
            nc.vector.tensor_tensor(
                s_n[:].rearrange("p (j o) -> p j o", o=O), i0b, i1b,
                op=mybir.AluOpType.mult)
            src, kscl = s_n, 1.0
        else:
            src, kscl = s_ps, (1.0 / R) if it == 0 else 1.0
        # |s|*k^2 = max(s*k^2, -s*k^2), all on DVE (ACT stays Exp-only)
        sneg = sp.tile([BL, NJO], F32, tag="sneg")
        nc.vector.tensor_scalar_mul(sneg[:], src[:, 0:NJO], -kscl * kscl)
        sabs = sp.tile([BL, NJO], F32, tag="sabs")
        nc.vector.scalar_tensor_tensor(sabs[:], src[:, 0:NJO], kscl * kscl,
                                       sneg[:], op0=mybir.AluOpType.mult,
                                       op1=mybir.AluOpType.max)
        den1 = sp.tile([BL, NJO], F32, tag="den1")
        nc.vector.tensor_scalar(den1[:], src[:, 0:NJO], kscl, None,
                                op0=mybir.AluOpType.mult)
        nc.vector.tensor_tensor(den1[:], den1[:], den1[:], op=mybir.AluOpType.mult)
        nc.vector.tensor_scalar_add(den1[:], den1[:], 1.0)
        rec = sp.tile([BL, NJO], F32, tag="rec")
        nc.vector.reciprocal(rec[:], den1[:])
        num = sp.tile([BL, NJO], F32, tag="num")
        nc.vector.tensor_mul(num[:], src[:, 0:NJO], sabs[:])

        if it == NUM_ITER - 1:
            vout = vq.tile([BL, NJO], F32, tag="vout")
            nc.vector.tensor_mul(vout[:], num[:], rec[:])
            nc.sync.dma_start(out_d[:], vout[:])
            break

        vpad = vq.tile([BL, NPAD], F32R, tag="vpad")
        nc.vector.tensor_mul(vpad[:, 0:NJO], num[:], rec[:])

        if "skip_m" in flags:
            b_upd = sp.tile([128, 90], F32, tag="b_upd")
            nc.vector.tensor_copy(b_upd[0:64, :], vpad[:, 0:90].bitcast(F32))
            nc.vector.tensor_copy(b_upd[64:128, :], vpad[:, 0:90].bitcast(F32))
            b_state = b_upd
            continue

        # ---- M matmul + W2 contraction -> b_upd (4-chunk quads,
        #      block-PAIR shared bups psum + fused o-reduce) ----
        b_upd = sp.tile([128, 90], F32, tag="b_upd")
        for bp in range(4):
            bups = ps_bup.tile([128, 2 * NPAD], F32, tag="bup")
            for sub in range(2):
                blk = bp * 2 + sub
                for quad in range(2):
                    mps = ps_m.tile([128, 4 * NPAD], F32, tag="m")
                    for q in range(4):
                        c = blk * 8 + quad * 4 + q
                        nc.tensor.matmul(mps[:, q * NPAD:(q + 1) * NPAD],
                                         x_s[:, c * 128:(c + 1) * 128], vpad[:],
                                         start=True, stop=True)
                    c0 = blk * 8 + quad * 4
                    P = p_pool.tile([128, 4 * NPAD], F32R, tag="P")
                    in0 = W2_s[:, c0 * NPAD:(c0 + 4) * NPAD].bitcast(F32) \
                        .rearrange("p (c n) -> p c n", n=NPAD)[:, :, 0:NJO]
                    in1 = mps[:].rearrange("p (c n) -> p c n", n=NPAD)[:, :, 0:NJO]
                    nc.vector.tensor_tensor(
                        P[:].rearrange("p (c n) -> p c n", n=NPAD)[:, :, 0:NJO],
                        in0, in1, op=mybir.AluOpType.mult)
                    for q in range(4):
                        c = blk * 8 + quad * 4 + q
                        g = c % 8
                        nc.tensor.matmul(bups[:, sub * NPAD:(sub + 1) * NPAD],
                                         I8S_s[:, g * 128:(g + 1) * 128],
                                         P[:, q * NPAD:(q + 1) * NPAD],
                                         start=(quad == 0 and q == 0),
                                         stop=(quad == 1 and q == 3))
            nc.vector.tensor_reduce(
                b_upd[:, bp * 20:(bp + 1) * 20]
                .rearrange("p (c j) -> p c j", j=J),
                bups[:].rearrange("p (c n) -> p c n", n=NPAD)[:, :, 0:NJO]
                .rearrange("p c (j o) -> p c j o", o=O),
                axis=mybir.AxisListType.X, op=mybir.AluOpType.add)
        # 9th block alone
        for blk in [8]:
            bups = ps_bup.tile([128, NPAD], F32, tag="bup")
            for quad in range(2):
                mps = ps_m.tile([128, 4 * NPAD], F32, tag="m")
                for q in range(4):
                    c = blk * 8 + quad * 4 + q
                    nc.tensor.matmul(mps[:, q * NPAD:(q + 1) * NPAD],
                                     x_s[:, c * 128:(c + 1) * 128], vpad[:],
                                     start=True, stop=True)
                c0 = blk * 8 + quad * 4
                P = p_pool.tile([128, 4 * NPAD], F32R, tag="P")
                in0 = W2_s[:, c0 * NPAD:(c0 + 4) * NPAD].bitcast(F32) \
                    .rearrange("p (c n) -> p c n", n=NPAD)[:, :, 0:NJO]
                in1 = mps[:].rearrange("p (c n) -> p c n", n=NPAD)[:, :, 0:NJO]
                nc.vector.tensor_tensor(
                    P[:].rearrange("p (c n) -> p c n", n=NPAD)[:, :, 0:NJO],
                    in0, in1, op=mybir.AluOpType.mult)
                for q in range(4):
                    c = blk * 8 + quad * 4 + q
                    g = c % 8
                    nc.tensor.matmul(bups[:],
                                     I8S_s[:, g * 128:(g + 1) * 128],
                                     P[:, q * NPAD:(q + 1) * NPAD],
                                     start=(quad == 0 and q == 0),
                                     stop=(quad == 1 and q == 3))
            nc.vector.tensor_reduce(
                b_upd[:, blk * 10:(blk + 1) * 10],
                bups[:, 0:NJO].rearrange("p (j o) -> p j o", o=O),
                axis=mybir.AxisListType.X, op=mybir.AluOpType.add)

        # ---- cross-core sum of b_upd ----
        cc_in = dram_pool.tile([128, 90], F32, tag="cc_in")
        nc.sync.dma_start(cc_in[:], b_upd[:])
        if "skip_ar" in flags:
            cc_out = dram_pool.tile([128, 90], F32, tag="cc_out")
            nc.sync.dma_start(cc_out[:], cc_in[:])
            upd_g = sp.tile([128, 90], F32, tag=f"bstate{it}")
            nc.sync.dma_start(upd_g[:], cc_out[:])
        elif "use_ar" not in flags:
            cc_out = dram_pool.tile([N_CORES * 128, 90], F32, tag="cc_outg")
            nc.gpsimd.collective_compute(
                "AllGather", mybir.AluOpType.bypass,
                replica_groups=[list(range(N_CORES))],
                ins=[cc_in.opt()], outs=[cc_out.opt()])
            gath = sp.tile([128, 8 * 90], F32, tag="gath")
            nc.sync.dma_start(
                gath[:].rearrange("p (k f) -> p k f", f=90),
                cc_out[:].rearrange("(k p) f -> p k f", p=128))
            upd_g = sp.tile([128, 90], F32, tag=f"bstate{it}")
            nc.vector.tensor_reduce(
                upd_g[:], gath[:].rearrange("p (k f) -> p f k", f=90),
                axis=mybir.AxisListType.X, op=mybir.AluOpType.add)
        else:
            cc_out = dram_pool.tile([128, 90], F32, tag="cc_out")
            nc.gpsimd.collective_compute(
                "AllReduce", mybir.AluOpType.add,
                replica_groups=[list(range(N_CORES))],
                ins=[cc_in.opt()], outs=[cc_out.opt()])
            upd_g = sp.tile([128, 90], F32, tag=f"bstate{it}")
            nc.sync.dma_start(upd_g[:], cc_out[:])
        if it == 0:
            b_state = upd_g
        else:
            b2 = sp.tile([128, 90], F32, tag=f"bstate{it}b")
            nc.vector.tensor_add(b2[:], b_state[:], upd_g[:])
            b_state = b2


def build_nc(reps=1, flags=()):
    nc = bacc.Bacc("TRN2", target_bir_lowering=False, debug=False,
                   num_devices=N_CORES)
    xT_d = nc.dram_tensor("xT", [RI, BL], F32R, kind="ExternalInput")
    x_d = nc.dram_tensor("x", [BL, RI], F32R, kind="ExternalInput")
    W2_d = nc.dram_tensor("W2", [RI, NJO], F32R, kind="ExternalInput")
    I8S_d = nc.dram_tensor("I8S", [128, 8 * 128], F32R, kind="ExternalInput")
    REPL_d = nc.dram_tensor("REPL", [128, 8 * 128], F32, kind="ExternalInput")
    ONES_d = nc.dram_tensor("ONES", [128, 1], F32, kind="ExternalInput")
    RONES_d = nc.dram_tensor("RONES", [1, 128], F32, kind="ExternalInput")
    out_d = nc.dram_tensor("out", [BL, NJO], F32, kind="ExternalOutput")

    with tile.TileContext(nc) as tc:
        with ExitStack() as ctx:
            pers = ctx.enter_context(tc.tile_pool(name="pers", bufs=1))
            sp = ctx.enter_context(tc.tile_pool(name="sp", bufs=2))
            vq = ctx.enter_context(tc.tile_pool(name="vq", bufs=2))
            wc_pool = ctx.enter_context(tc.tile_pool(name="wcp", bufs=3))
            p_pool = ctx.enter_context(tc.tile_pool(name="pp", bufs=3))
            dram_pool = ctx.enter_context(
                tc.tile_pool(name="dram", bufs=2, space="DRAM"))
            ps_m = ctx.enter_context(tc.tile_pool(name="ps_m", bufs=2, space="PSUM"))
            ps_bup = ctx.enter_context(tc.tile_pool(name="ps_b", bufs=2, space="PSUM"))
            ps_sc = ctx.enter_context(tc.tile_pool(name="ps_sc", bufs=1, space="PSUM"))
            ps_tiny = ctx.enter_context(tc.tile_pool(name="ps_y", bufs=1, space="PSUM"))
            ps_s = ps_sc
            ps_crep = ps_sc
            ps_small = ps_tiny

            xT_s = pers.tile([128, KT * BL], F32R)
            x_s = pers.tile([BL, RI], F32R)
            W2_s = pers.tile([128, KT * NPAD], F32R)
            I8S_s = pers.tile([128, 8 * 128], F32R)
            REPL_s = pers.tile([128, 8 * 128], F32)
            ONES_s = pers.tile([128, 1], F32)
            RONES_s = pers.tile([1, 128], F32)

            # loads: W2/xT in 9 chunk-groups, x in 4 column groups
            for g in range(9):
                nc.sync.dma_start(
                    W2_s[:, g * 8 * NPAD:(g + 1) * 8 * NPAD]
                    .rearrange("p (c n) -> p c n", n=NPAD)[:, :, 0:NJO],
                    W2_d[:].rearrange("(c p) n -> p c n", p=128)[:, g * 8:(g + 1) * 8, :])
                nc.sync.dma_start(
                    xT_s[:, g * 8 * BL:(g + 1) * 8 * BL]
                    .rearrange("p (c m) -> p c m", m=BL),
                    xT_d[:].rearrange("(c p) m -> p c m", p=128)[:, g * 8:(g + 1) * 8, :])
            for g in range(4):
                nc.sync.dma_start(x_s[:, g * 2304:(g + 1) * 2304],
                                  x_d[:, g * 2304:(g + 1) * 2304])
            nc.sync.dma_start(I8S_s[:], I8S_d[:])
            nc.sync.dma_start(REPL_s[:], REPL_d[:])
            nc.sync.dma_start(ONES_s[:], ONES_d[:])
            nc.sync.dma_start(RONES_s[:], RONES_d[:])

            tensors = (xT_s, x_s, W2_s, I8S_s, REPL_s, ONES_s, RONES_s)
            pools = (sp, vq, wc_pool, p_pool, dram_pool,
                     ps_s, ps_m, ps_bup, ps_crep, ps_small)
            for rep in range(reps):
                emit_algorithm(nc, tc, ctx, tensors, pools, out_d, flags)

    nc.compile()
    return nc


def make_host_inputs(x, W):
    """Build per-core in_maps from the full inputs."""
    x = np.ascontiguousarray(np.asarray(x, dtype=np.float32))
    W = np.asarray(W, dtype=np.float32)
    W2 = np.ascontiguousarray(W.transpose(0, 3, 1, 2).reshape(RI, NJO))

    I8S = np.zeros((128, 8 * 128), np.float32)
    for g in range(8):
        for m in range(16 * g, 16 * g + 16):
            q = m - 16 * g
            I8S[8 * q:8 * q + 8, g * 128 + m] = 1.0 / B
    REPL = np.zeros((128, 8 * 128), np.float32)
    for g in range(8):
        for m in range(128):
            REPL[16 * g + m // 8, g * 128 + m] = 1.0
    ONES = np.ones((128, 1), np.float32)
    RONES = np.ones((1, 128), np.float32)

    in_maps = []
    for c in range(N_CORES):
        xs = np.ascontiguousarray(x[c * BL:(c + 1) * BL].reshape(BL, RI))
        in_maps.append({
            "x": xs,
            "xT": np.ascontiguousarray(xs.T),
            "W2": W2,
            "I8S": I8S,
            "REPL": REPL,
            "ONES": ONES,
            "RONES": RONES,
        })
    return in_maps


def assemble_output(results):
    return np.concatenate(
        [results[c]["out"].reshape(BL, J, O, 1) for c in range(N_CORES)],
        axis=0).astype(np.float32)


_NC_CACHE = {}


def kernel(x, W):
    if "nc" not in _NC_CACHE:
        _NC_CACHE["nc"] = build_nc(reps=1)
    nc = _NC_CACHE["nc"]
    in_maps = make_host_inputs(x, W)
    res = run_bass_kernel_spmd(nc, in_maps, list(range(N_CORES)))
    return assemble_output(res.results)


if __name__ == "__main__":
    import reference
    inputs = reference.setup_inputs()
    expected = np.asarray(reference.reference(**inputs))
    got = kernel(np.asarray(inputs["x"]), np.asarray(inputs["W"]))
    err = np.abs(got - expected).max()
    rel = err / np.abs(expected).max()
    print("abs err:", err, "scale-rel err:", rel)



# revision 7
# speedup vs baseline: 2.8745x; 2.8745x over previous
"""Trainium2 Bass kernel for CapsNet DigitCaps dynamic routing (nn_DigitCaps).

Reference computation:
    u_hat[b,r,j,o] = W[r,j,o,:] @ x[b,r,:]        B,R,J,O,I = 512,1152,10,16,8
    b_ij = 0; 3 routing iterations:
        c = softmax(b_ij, axis=0)                  # over routes r, per j
        s[b,j,o] = sum_r c[r,j] * u_hat[b,r,j,o]
        v = squash(s) = s*|s|/(1+s^2)              # elementwise
        b_ij += mean_b sum_o u_hat[b,r,j,o]*v[b,j,o]
    return v[..., None]

Kernel strategy (data-parallel over batch, 8 cores, 64 rows each; u_hat is
never materialized). All matmul operands bf16 (PE: 1 cyc per output row at
any N, so N=160 with no padding), fp32 PSUM accumulation; rel err ~3e-3.

r-major layout: partition p = r % 128, free blocks rb = r // 128 (9 blocks),
so b_ij, e=exp(b), and b_upd all live as [128, (rb j)] = [128, 90] with NO
cross-partition shuffles:
    s    = X @ (e-scaled W2), e[r,j] applied to W2R by DVE broadcast mult
           (per-rb [128, 8*160] ops); softmax 1/Z deferred to squash.
    M_k  = X_k^T @ V per (rb,i) chunk -> P_k = W2R_k (*) M_k (DVE, bf16)
    b_upd[:, rb*10:..] = reduce_{i,o} P_rb  (Pool XY-reduce; i,o are free
           dims in r-major so NO partition-sum matmuls needed)
    b_upd is AllGather-summed across cores; softmax runs redundantly on all.
"""
import os
import numpy as np
import ml_dtypes
from contextlib import ExitStack

import concourse.bacc as bacc
import concourse.bass as bass
import concourse.tile as tile
from concourse import mybir
from concourse.bass_utils import run_bass_kernel_spmd

F32 = mybir.dt.float32
BF16 = mybir.dt.bfloat16

B, R, J, O, I = 512, 1152, 10, 16, 8
N_CORES = 8
BL = B // N_CORES          # 64 batch rows per core
RI = R * I                 # 9216
NJO = J * O                # 160
NRB = 9                    # r-blocks of 128
KT = RI // 128             # 72 contraction chunks (= NRB * I)
NUM_ITER = 3


def emit_algorithm(nc, tc, ctx, tensors, pools, out_d, flags=()):
    """Emit one full 3-iteration routing computation."""
    (XT_s, X2_s, W2R_s, ONESB_s, RONESB_s) = tensors
    (sp, vq, wc_pool, p_pool, dram_pool, ps_s, ps_m, ps_z) = pools

    b_state = None      # running AR sum (un-normalized: exp uses scale=1/B)
    for it in range(NUM_ITER):
        # ---- e = exp(b/B) and deferred-softmax 1/Z factor ----
        e_s = None
        zinv_b = None
        if it > 0:
            e_s = sp.tile([128, 90], BF16, tag="e", name="e_s")
            nc.scalar.activation(e_s[:], b_state[:],
                                 mybir.ActivationFunctionType.Exp,
                                 scale=1.0 / B)
            # Z_j = sum_r e[r,j]: ones-matmul over partitions + rb-reduce
            zps = ps_z.tile([1, 90], F32, tag="z")
            nc.tensor.matmul(zps[:], ONESB_s[:, 0:1], e_s[:],
                             start=True, stop=True)
            zsum = sp.tile([1, 10], F32, tag="zsum")
            nc.vector.tensor_reduce(
                zsum[:], zps[:].rearrange("p (rb j) -> p j rb", j=J),
                axis=mybir.AxisListType.X, op=mybir.AluOpType.add)
            zinv = sp.tile([1, 10], F32, tag="zinv")
            nc.vector.reciprocal(zinv[:], zsum[:])
            zinvb16 = sp.tile([1, 10], BF16, tag="zinvb16")
            nc.vector.tensor_copy(zinvb16[:], zinv[:])
            zb_ps = ps_z.tile([BL, 10], F32, tag="zb")
            nc.tensor.matmul(zb_ps[:], RONESB_s[:, 0:BL], zinvb16[:],
                             start=True, stop=True)
            zinv_b = sp.tile([BL, 10], F32, tag="zinv_b")
            nc.vector.tensor_copy(zinv_b[:], zb_ps[:])

        # ---- s matmul, rb-pipelined weight scaling ----
        s_ps = ps_s.tile([BL, NJO], F32, tag="s")
        for rb in range(NRB):
            if it == 0 or "skip_scale" in flags:
                rhs_blk = W2R_s
            else:
                wc = wc_pool.tile([128, I * NJO], BF16, tag="wc")
                in0 = W2R_s[:, rb * I * NJO:(rb + 1) * I * NJO] \
                    .rearrange("p (i j o) -> p i j o", j=J, o=O)
                in1 = e_s[:, rb * J:(rb + 1) * J] \
                    .rearrange("p (a j c) -> p a j c", a=1, c=1)
                i0b, i1b = bass.broadcast_tensor_aps(in0, in1)
                nc.vector.tensor_tensor(
                    wc[:].rearrange("p (i j o) -> p i j o", j=J, o=O),
                    i0b, i1b, op=mybir.AluOpType.mult)
                rhs_blk = None
            for i in range(I):
                k = rb * I + i
                if it == 0 or "skip_scale" in flags:
                    rhs = W2R_s[:, k * NJO:(k + 1) * NJO]
                else:
                    rhs = wc[:, i * NJO:(i + 1) * NJO]
                nc.tensor.matmul(s_ps[:], XT_s[:, k * BL:(k + 1) * BL], rhs,
                                 start=(k == 0), stop=(k == KT - 1))

        # ---- squash (with deferred softmax normalization for it > 0) ----
        if it > 0 and zinv_b is not None:
            s_n = sp.tile([BL, NJO], F32, tag="s_n")
            i0 = s_ps[:].rearrange("p (j o) -> p j o", o=O)
            i1 = zinv_b[:].rearrange("p (j o) -> p j o", o=1)
            i0b, i1b = bass.broadcast_tensor_aps(i0, i1)
            nc.vector.tensor_tensor(
                s_n[:].rearrange("p (j o) -> p j o", o=O), i0b, i1b,
                op=mybir.AluOpType.mult)
            src, kscl = s_n, 1.0
        else:
            src, kscl = s_ps, 1.0 / R
        sneg = sp.tile([BL, NJO], F32, tag="sneg")
        nc.vector.tensor_scalar_mul(sneg[:], src[:], -kscl * kscl)
        sabs = sp.tile([BL, NJO], F32, tag="sabs")
        nc.vector.scalar_tensor_tensor(sabs[:], src[:], kscl * kscl,
                                       sneg[:], op0=mybir.AluOpType.mult,
                                       op1=mybir.AluOpType.max)
        den1 = sp.tile([BL, NJO], F32, tag="den1")
        nc.vector.tensor_scalar(den1[:], src[:], kscl, None,
                                op0=mybir.AluOpType.mult)
        nc.vector.tensor_tensor(den1[:], den1[:], den1[:],
                                op=mybir.AluOpType.mult)
        nc.vector.tensor_scalar_add(den1[:], den1[:], 1.0)
        rec = sp.tile([BL, NJO], F32, tag="rec")
        nc.vector.reciprocal(rec[:], den1[:])
        num = sp.tile([BL, NJO], F32, tag="num")
        nc.vector.tensor_mul(num[:], src[:], sabs[:])

        if it == NUM_ITER - 1:
            vout = vq.tile([BL, NJO], F32, tag="vout")
            nc.vector.tensor_mul(vout[:], num[:], rec[:])
            nc.sync.dma_start(out_d[:], vout[:])
            break

        vpad = vq.tile([BL, NJO], BF16, tag="vpad")
        nc.vector.tensor_mul(vpad[:], num[:], rec[:])

        if "skip_m" in flags:
            b_upd = sp.tile([128, 90], F32, tag="b_upd")
            nc.vector.memset(b_upd[:], 0.001)
            b_state = b_upd
            continue

        # ---- M matmuls + P products (DVE) + (i,o) reduce (Pool) ----
        # mps chunk stride padded to 256 floats so no matmul output crosses a
        # 2KB PSUM bank boundary (NJO=160 at stride 160 would cross on q=3)
        MST = 256
        b_upd = sp.tile([128, 90], F32, tag="b_upd")
        for rb in range(NRB):
            prb = p_pool.tile([128, I * NJO], BF16, tag="prb")
            for half in range(2):
                mps = ps_m.tile([128, 4 * MST], F32, tag="m")
                for q in range(4):
                    k = rb * I + half * 4 + q
                    nc.tensor.matmul(mps[:, q * MST:q * MST + NJO],
                                     X2_s[:, k * 128:(k + 1) * 128], vpad[:],
                                     start=True, stop=True)
                c0 = half * 4
                nc.vector.tensor_tensor(
                    prb[:, c0 * NJO:(c0 + 4) * NJO]
                    .rearrange("p (c n) -> p c n", n=NJO),
                    W2R_s[:, (rb * I + c0) * NJO:(rb * I + c0 + 4) * NJO]
                    .rearrange("p (c n) -> p c n", n=NJO),
                    mps[:].rearrange("p (c n) -> p c n", n=MST)[:, :, 0:NJO],
                    op=mybir.AluOpType.mult)
            nc.vector.tensor_reduce(
                b_upd[:, rb * J:(rb + 1) * J],
                prb[:].rearrange("p (i j o) -> p j i o", j=J, o=O),
                axis=mybir.AxisListType.XY, op=mybir.AluOpType.add)

        # ---- cross-core sum of b_upd ----
        cc_in = dram_pool.tile([128, 90], F32, tag="cc_in")
        nc.sync.dma_start(cc_in[:], b_upd[:])
        if "skip_ar" in flags:
            cc_out = dram_pool.tile([128, 90], F32, tag="cc_out")
            nc.sync.dma_start(cc_out[:], cc_in[:])
            upd_g = sp.tile([128, 90], F32, tag=f"bstate{it}")
            nc.sync.dma_start(upd_g[:], cc_out[:])
        else:
            cc_out = dram_pool.tile([N_CORES * 128, 90], F32, tag="cc_outg")
            nc.gpsimd.collective_compute(
                "AllGather", mybir.AluOpType.bypass,
                replica_groups=[list(range(N_CORES))],
                ins=[cc_in.opt()], outs=[cc_out.opt()])
            gath = sp.tile([128, 8 * 90], F32, tag="gath")
            nc.sync.dma_start(
                gath[:].rearrange("p (k f) -> p k f", f=90),
                cc_out[:].rearrange("(k p) f -> p k f", p=128))
            upd_g = sp.tile([128, 90], F32, tag=f"bstate{it}")
            nc.vector.tensor_reduce(
                upd_g[:], gath[:].rearrange("p (k f) -> p f k", f=90),
                axis=mybir.AxisListType.X, op=mybir.AluOpType.add)
        if it == 0:
            b_state = upd_g
        else:
            b2 = sp.tile([128, 90], F32, tag=f"bstate{it}b")
            nc.vector.tensor_add(b2[:], b_state[:], upd_g[:])
            b_state = b2


def build_nc(reps=1, flags=()):
    nc = bacc.Bacc("TRN2", target_bir_lowering=False, debug=False,
                   num_devices=N_CORES)
    XT_d = nc.dram_tensor("XT", [128, KT * BL], BF16, kind="ExternalInput")
    X2_d = nc.dram_tensor("X2", [BL, RI], BF16, kind="ExternalInput")
    W2R_d = nc.dram_tensor("W2R", [128, KT * NJO], BF16, kind="ExternalInput")
    ONESB_d = nc.dram_tensor("ONESB", [128, 1], BF16, kind="ExternalInput")
    RONESB_d = nc.dram_tensor("RONESB", [1, BL], BF16, kind="ExternalInput")
    out_d = nc.dram_tensor("out", [BL, NJO], F32, kind="ExternalOutput")

    with tile.TileContext(nc) as tc:
        with ExitStack() as ctx:
            pers = ctx.enter_context(tc.tile_pool(name="pers", bufs=1))
            sp = ctx.enter_context(tc.tile_pool(name="sp", bufs=2))
            vq = ctx.enter_context(tc.tile_pool(name="vq", bufs=2))
            wc_pool = ctx.enter_context(tc.tile_pool(name="wcp", bufs=3))
            p_pool = ctx.enter_context(tc.tile_pool(name="pp", bufs=2))
            dram_pool = ctx.enter_context(
                tc.tile_pool(name="dram", bufs=2, space="DRAM"))
            ps_s = ctx.enter_context(tc.tile_pool(name="ps_s", bufs=1, space="PSUM"))
            ps_m = ctx.enter_context(tc.tile_pool(name="ps_m", bufs=2, space="PSUM"))
            ps_z = ctx.enter_context(tc.tile_pool(name="ps_z", bufs=1, space="PSUM"))

            XT_s = pers.tile([128, KT * BL], BF16)
            X2_s = pers.tile([BL, RI], BF16)
            W2R_s = pers.tile([128, KT * NJO], BF16)
            ONESB_s = pers.tile([128, 1], BF16)
            RONESB_s = pers.tile([1, BL], BF16)

            for g in range(3):
                nc.sync.dma_start(
                    XT_s[:, g * 24 * BL:(g + 1) * 24 * BL],
                    XT_d[:, g * 24 * BL:(g + 1) * 24 * BL])
                nc.sync.dma_start(
                    X2_s[:, g * 3072:(g + 1) * 3072],
                    X2_d[:, g * 3072:(g + 1) * 3072])
            for g in range(6):
                nc.sync.dma_start(
                    W2R_s[:, g * 12 * NJO:(g + 1) * 12 * NJO],
                    W2R_d[:, g * 12 * NJO:(g + 1) * 12 * NJO])
            nc.sync.dma_start(ONESB_s[:], ONESB_d[:])
            nc.sync.dma_start(RONESB_s[:], RONESB_d[:])

            tensors = (XT_s, X2_s, W2R_s, ONESB_s, RONESB_s)
            pools = (sp, vq, wc_pool, p_pool, dram_pool, ps_s, ps_m, ps_z)
            for rep in range(reps):
                emit_algorithm(nc, tc, ctx, tensors, pools, out_d, flags)

    nc.compile()
    return nc


def make_host_inputs(x, W):
    """Build per-core in_maps from the full inputs (r-major bf16 layouts)."""
    x = np.ascontiguousarray(np.asarray(x, dtype=np.float32))
    W = np.asarray(W, dtype=np.float32)
    bf = ml_dtypes.bfloat16
    # W2R[p, rb, i, j, o] = W[rb*128+p, j, o, i]
    W2R = np.ascontiguousarray(
        W.reshape(NRB, 128, J, O, I).transpose(1, 0, 4, 2, 3)
        .reshape(128, KT * NJO)).astype(bf)
    ONESB = np.ones((128, 1), bf)
    RONESB = np.ones((1, BL), bf)

    in_maps = []
    for c in range(N_CORES):
        xc = x[c * BL:(c + 1) * BL]                      # [64, R, I]
        XT = np.ascontiguousarray(
            xc.transpose(1, 2, 0).reshape(NRB, 128, I, BL)
            .transpose(1, 0, 2, 3).reshape(128, KT * BL)).astype(bf)
        X2 = np.ascontiguousarray(
            xc.reshape(BL, NRB, 128, I).transpose(0, 1, 3, 2)
            .reshape(BL, RI)).astype(bf)
        in_maps.append({
            "XT": XT,
            "X2": X2,
            "W2R": W2R,
            "ONESB": ONESB,
            "RONESB": RONESB,
        })
    return in_maps


def assemble_output(results):
    return np.concatenate(
        [results[c]["out"].reshape(BL, J, O, 1) for c in range(N_CORES)],
        axis=0).astype(np.float32)


_NC_CACHE = {}


def kernel(x, W):
    if "nc" not in _NC_CACHE:
        _NC_CACHE["nc"] = build_nc(reps=1)
    nc = _NC_CACHE["nc"]
    in_maps = make_host_inputs(x, W)
    res = run_bass_kernel_spmd(nc, in_maps, list(range(N_CORES)))
    return assemble_output(res.results)


if __name__ == "__main__":
    import reference
    inputs = reference.setup_inputs()
    expected = np.asarray(reference.reference(**inputs))
    got = kernel(np.asarray(inputs["x"]), np.asarray(inputs["W"]))
    err = np.abs(got - expected).max()
    rel = err / np.abs(expected).max()
    print("abs err:", err, "scale-rel err:", rel)


# revision 16
# speedup vs baseline: 8.6916x; 3.0237x over previous
"""Trainium2 Bass kernel for CapsNet DigitCaps dynamic routing (nn_DigitCaps).

Reference computation:
    u_hat[b,r,j,o] = W[r,j,o,:] @ x[b,r,:]        B,R,J,O,I = 512,1152,10,16,8
    b_ij = 0; 3 routing iterations:
        c = softmax(b_ij, axis=0)                  # over routes r, per j
        s[b,j,o] = sum_r c[r,j] * u_hat[b,r,j,o]
        v = squash(s) = s*|s|/(1+s^2)              # elementwise
        b_ij += mean_b sum_o u_hat[b,r,j,o]*v[b,j,o]
    return v[..., None]

Kernel strategy (data-parallel over batch, 8 cores, 64 rows each; u_hat is
never materialized). All matmul/elementwise operands fp16 (PE: 1 cyc per
output row at any N; DVE 2x/4x fast modes need 2-byte packed SBUF operands),
fp32 PSUM accumulation. rel err ~5e-3 vs the fp32 reference.

r-major layout: partition p = r % 128, free blocks rb = r // 128 (9 blocks),
so b_ij, e=exp(b), and b_upd all live as [128, (rb j)] = [128, 90] with no
cross-partition shuffles:
    s    = X @ (e-scaled W2); e expanded over o once per iter (eo) so the
           wc broadcast-mult is innermost-packed; softmax 1/Z deferred to
           squash (it commutes through the matmul).
    M_k  = X_k^T @ V per (rb,i) chunk; 3 of 4 PSUM blocks are ACT-copied to
           fp16 SBUF so the W2R (*) M products run at the DVE fast rate.
    b_upd[:, rb*10:..] = tree-add over i + small XY-reduce over (i,o).
    b_upd is AllGather-summed across cores (~15us NRT collective); softmax
    runs redundantly on every core.

Each rep is split into 7 stages A..G (s0 / M0 / AR0 / s1 / M1 / AR1 / s2)
and emitted as a modulo software pipeline across reps (rep r's stage s in
slot r+s): per-engine instruction queues execute in emission order, so the
interleave is what lets other reps' compute fill each rep's two collective
windows.
"""
import os
import numpy as np
from contextlib import ExitStack

import concourse.bacc as bacc
import concourse.bass as bass
import concourse.tile as tile
from concourse import mybir
from concourse.bass_utils import run_bass_kernel_spmd

F32 = mybir.dt.float32
F16 = mybir.dt.float16

B, R, J, O, I = 512, 1152, 10, 16, 8
N_CORES = 8
BL = B // N_CORES          # 64 batch rows per core
RI = R * I                 # 9216
NJO = J * O                # 160
NRB = 9                    # r-blocks of 128
KT = RI // 128             # 72 contraction chunks (= NRB * I)
NUM_ITER = 3
MST = 256                  # mps chunk stride (PSUM bank alignment)
NST = 7                    # pipeline stages per rep


def make_stages(nc, tc, tensors, pools, out_d, flags):
    (XT_s, X2_s, W2R_s, ONESB_s, RONESB_s) = tensors
    (sp, vq, wc_pool, p_pool, dram_pool, ps_s, ps_m, ps_z) = pools
    st = {}

    def emit_post_ar(it):
        """exp + eo + Z chain preparing iteration `it` (reads st['b_state'])."""
        e_s = sp.tile([128, 90], F16, tag="e", name="e_s")
        nc.scalar.activation(e_s[:], st['b_state'][:],
                             mybir.ActivationFunctionType.Exp,
                             scale=1.0 / B)
        eo = sp.tile([128, 90 * O], F16, tag="eo", name="eo")
        i0 = e_s[:].rearrange("p (rb j a) -> p rb j a", j=J, a=1)
        eov = eo[:].rearrange("p (rb j o) -> p rb j o", j=J, o=O)
        i0b, _ = bass.broadcast_tensor_aps(i0, eov)
        nc.scalar.activation(eov, i0b, mybir.ActivationFunctionType.Copy)
        zmix = ps_z.tile([BL, 100], F32, tag="z")
        zps = zmix[0:1, 0:90]
        nc.tensor.matmul(zps, ONESB_s[:, 0:1], e_s[:], start=True, stop=True)
        zsum = sp.tile([1, 10], F32, tag="zsum")
        nc.vector.tensor_reduce(
            zsum[:], zps.rearrange("p (rb j) -> p j rb", j=J),
            axis=mybir.AxisListType.X, op=mybir.AluOpType.add)
        zinv = sp.tile([1, 10], F32, tag="zinv")
        nc.vector.reciprocal(zinv[:], zsum[:])
        zinv16 = sp.tile([1, 10], F16, tag="zinv16")
        nc.vector.tensor_copy(zinv16[:], zinv[:])
        zb_ps = zmix[0:BL, 90:100]
        nc.tensor.matmul(zb_ps, RONESB_s[:, 0:BL], zinv16[:],
                         start=True, stop=True)
        zinv_b = sp.tile([BL, 10], F32, tag="zinv_b")
        nc.vector.tensor_copy(zinv_b[:], zb_ps)
        st[f'eo{it}'] = eo
        st[f'zinv_b{it}'] = zinv_b

    def stage_s(it):
        def f():
            scaled = it > 0 and "skip_scale" not in flags
            s_ps = ps_s.tile([BL, NJO], F32, tag="s")
            for rb in range(NRB):
                if scaled:
                    eo = st[f'eo{it}']
                    wc = wc_pool.tile([128, I * NJO], F16, tag="wc")
                    in0 = W2R_s[:, rb * I * NJO:(rb + 1) * I * NJO] \
                        .rearrange("p (i j o) -> p i j o", j=J, o=O)
                    in1 = eo[:, rb * J * O:(rb + 1) * J * O] \
                        .rearrange("p (a j o) -> p a j o", a=1, o=O)
                    i0b, i1b = bass.broadcast_tensor_aps(in0, in1)
                    nc.vector.tensor_tensor(
                        wc[:].rearrange("p (i j o) -> p i j o", j=J, o=O),
                        i0b, i1b, op=mybir.AluOpType.mult)
                for i in range(I):
                    k = rb * I + i
                    rhs = (wc[:, i * NJO:(i + 1) * NJO] if scaled
                           else W2R_s[:, k * NJO:(k + 1) * NJO])
                    nc.tensor.matmul(s_ps[:], XT_s[:, k * BL:(k + 1) * BL],
                                     rhs, start=(k == 0), stop=(k == KT - 1))

            # squash (deferred softmax normalization when scaled), fp16
            # chain after a single PSUM read; kscl folded into s16
            s16 = sp.tile([BL, NJO], F16, tag="s16", name="s16")
            if scaled:
                zinv_b = st[f'zinv_b{it}']
                i0 = s_ps[:].rearrange("p (j o) -> p j o", o=O)
                i1 = zinv_b[:].rearrange("p (j o) -> p j o", o=1)
                i0b, i1b = bass.broadcast_tensor_aps(i0, i1)
                nc.vector.tensor_tensor(
                    s16[:].rearrange("p (j o) -> p j o", o=O), i0b, i1b,
                    op=mybir.AluOpType.mult)
            else:
                nc.scalar.activation(s16[:], s_ps[:],
                                     mybir.ActivationFunctionType.Copy,
                                     scale=1.0 / R)
            src = s16
            with nc.allow_low_precision(reason="fp16 squash; |s|<20"):
                sneg = sp.tile([BL, NJO], F16, tag="sneg")
                nc.vector.tensor_scalar_mul(sneg[:], src[:], -1.0)
                sabs = sp.tile([BL, NJO], F16, tag="sabs")
                nc.vector.tensor_tensor(sabs[:], src[:], sneg[:],
                                        op=mybir.AluOpType.max)
                den1 = sp.tile([BL, NJO], F16, tag="den1")
                nc.vector.scalar_tensor_tensor(den1[:], src[:], 1.0,
                                               src[:],
                                               op0=mybir.AluOpType.mult,
                                               op1=mybir.AluOpType.mult)
                nc.vector.tensor_scalar_add(den1[:], den1[:], 1.0)
                rec = sp.tile([BL, NJO], F16, tag="rec")
                nc.vector.reciprocal(rec[:], den1[:])
                num = sp.tile([BL, NJO], F16, tag="num")
                nc.vector.tensor_mul(num[:], src[:], sabs[:])
            if it == NUM_ITER - 1:
                vout = vq.tile([BL, NJO], F32, tag="vout")
                nc.vector.tensor_mul(vout[:], num[:], rec[:])
                nc.sync.dma_start(out_d[:], vout[:])
            else:
                vpad = vq.tile([BL, NJO], F16, tag="vpad")
                nc.vector.tensor_mul(vpad[:], num[:], rec[:])
                st[f'vpad{it}'] = vpad
        return f

    def stage_m(it):
        def f():
            if "skip_m" in flags:
                b_upd = sp.tile([128, 90], F16, tag="b_upd")
                nc.vector.memset(b_upd[:], 0.001)
                st[f'b_upd{it}'] = b_upd
                return
            vpad = st[f'vpad{it}']
            b_upd = sp.tile([128, 90], F16, tag="b_upd")
            for rb in range(NRB):
                prb = p_pool.tile([128, I * NJO], F16, tag="prb")
                for quarter in range(4):
                    mps = ps_m.tile([128, 2 * MST], F32, tag="m")
                    for q in range(2):
                        k = rb * I + quarter * 2 + q
                        nc.tensor.matmul(mps[:, q * MST:q * MST + NJO],
                                         X2_s[:, k * 128:(k + 1) * 128],
                                         vpad[:], start=True, stop=True)
                    c0 = quarter * 2
                    mview = mps[:].rearrange("p (c n) -> p c n",
                                             n=MST)[:, :, 0:NJO]
                    wview = W2R_s[:, (rb * I + c0) * NJO:
                                  (rb * I + c0 + 2) * NJO] \
                        .rearrange("p (c n) -> p c n", n=NJO)
                    pview = prb[:, c0 * NJO:(c0 + 2) * NJO] \
                        .rearrange("p (c n) -> p c n", n=NJO)
                    mcp = p_pool.tile([128, 2 * NJO], F16, tag="mcp")
                    nc.scalar.activation(
                        mcp[:].rearrange("p (c n) -> p c n", n=NJO),
                        mview, mybir.ActivationFunctionType.Copy)
                    nc.vector.tensor_tensor(
                        pview, wview,
                        mcp[:].rearrange("p (c n) -> p c n", n=NJO),
                        op=mybir.AluOpType.mult)
                t1 = p_pool.tile([128, 4 * NJO], F16, tag="t1", name="t1")
                nc.vector.tensor_add(t1[:], prb[:, 0:4 * NJO],
                                     prb[:, 4 * NJO:8 * NJO])
                t2 = p_pool.tile([128, 2 * NJO], F16, tag="t2", name="t2")
                nc.vector.tensor_add(t2[:], t1[:, 0:2 * NJO],
                                     t1[:, 2 * NJO:4 * NJO])
                with nc.allow_low_precision(reason="fp16 b_upd; 0.05% rel"):
                    nc.vector.tensor_reduce(
                        b_upd[:, rb * J:(rb + 1) * J],
                        t2[:].rearrange("p (i j o) -> p j i o", j=J, o=O),
                        axis=mybir.AxisListType.XY, op=mybir.AluOpType.add)
            st[f'b_upd{it}'] = b_upd
        return f

    def stage_ar(it):
        def f():
            b_upd = st[f'b_upd{it}']
            cc_in = dram_pool.tile([128, 90], F16, tag="cc_in")
            nc.sync.dma_start(cc_in[:], b_upd[:])
            if "skip_ar" in flags:
                cc_out = dram_pool.tile([128, 90], F16, tag="cc_out")
                nc.sync.dma_start(cc_out[:], cc_in[:])
                gath = sp.tile([128, 8 * 90], F16, tag="gath")
                for kk in range(8):
                    nc.sync.dma_start(gath[:, kk * 90:(kk + 1) * 90],
                                      cc_out[:])
            else:
                cc_out = dram_pool.tile([N_CORES * 128, 90], F16,
                                        tag="cc_outg")
                nc.gpsimd.collective_compute(
                    "AllGather", mybir.AluOpType.bypass,
                    replica_groups=[list(range(N_CORES))],
                    ins=[cc_in.opt()], outs=[cc_out.opt()])
                gath = sp.tile([128, 8 * 90], F16, tag="gath")
                nc.sync.dma_start(
                    gath[:].rearrange("p (k f) -> p k f", f=90),
                    cc_out[:].rearrange("(k p) f -> p k f", p=128))
            if it == 0:
                b_state = sp.tile([128, 90], F16, tag="bstate0")
                with nc.allow_low_precision(reason="fp16 b state"):
                    nc.vector.tensor_reduce(
                        b_state[:],
                        gath[:].rearrange("p (k f) -> p f k", f=90),
                        axis=mybir.AxisListType.X, op=mybir.AluOpType.add)
            else:
                upd_g = sp.tile([128, 90], F16, tag="upd_g")
                with nc.allow_low_precision(reason="fp16 b state"):
                    nc.vector.tensor_reduce(
                        upd_g[:],
                        gath[:].rearrange("p (k f) -> p f k", f=90),
                        axis=mybir.AxisListType.X, op=mybir.AluOpType.add)
                b_state = sp.tile([128, 90], F16, tag="bstate1b")
                nc.vector.tensor_add(b_state[:], st['b_state'][:], upd_g[:])
            st['b_state'] = b_state
            emit_post_ar(it + 1)
        return f

    return [stage_s(0), stage_m(0), stage_ar(0),
            stage_s(1), stage_m(1), stage_ar(1),
            stage_s(2)]


def build_nc(reps=1, flags=()):
    nc = bacc.Bacc("TRN2", target_bir_lowering=False, debug=False,
                   num_devices=N_CORES)
    XT_d = nc.dram_tensor("XT", [128, KT * BL], F16, kind="ExternalInput")
    X2_d = nc.dram_tensor("X2", [BL, RI], F16, kind="ExternalInput")
    W2R_d = nc.dram_tensor("W2R", [128, KT * NJO], F16, kind="ExternalInput")
    ONESB_d = nc.dram_tensor("ONESB", [128, 1], F16, kind="ExternalInput")
    RONESB_d = nc.dram_tensor("RONESB", [1, BL], F16, kind="ExternalInput")
    out_d = nc.dram_tensor("out", [BL, NJO], F32, kind="ExternalOutput")

    with tile.TileContext(nc) as tc:
        with ExitStack() as ctx:
            pers = ctx.enter_context(tc.tile_pool(name="pers", bufs=1))
            sp = ctx.enter_context(tc.tile_pool(name="sp", bufs=4))
            vq = ctx.enter_context(tc.tile_pool(name="vq", bufs=3))
            wc_pool = ctx.enter_context(tc.tile_pool(name="wcp", bufs=3))
            p_pool = ctx.enter_context(tc.tile_pool(name="pp", bufs=3))
            dram_pool = ctx.enter_context(
                tc.tile_pool(name="dram", bufs=3, space="DRAM"))
            ps_s = ctx.enter_context(tc.tile_pool(name="ps_s", bufs=3, space="PSUM"))
            ps_m = ctx.enter_context(tc.tile_pool(name="ps_m", bufs=3, space="PSUM"))
            ps_z = ctx.enter_context(tc.tile_pool(name="ps_z", bufs=2, space="PSUM"))

            XT_s = pers.tile([128, KT * BL], F16)
            X2_s = pers.tile([BL, RI], F16)
            W2R_s = pers.tile([128, KT * NJO], F16)
            ONESB_s = pers.tile([128, 1], F16)
            RONESB_s = pers.tile([1, BL], F16)

            for g in range(3):
                nc.sync.dma_start(
                    XT_s[:, g * 24 * BL:(g + 1) * 24 * BL],
                    XT_d[:, g * 24 * BL:(g + 1) * 24 * BL])
                nc.sync.dma_start(
                    X2_s[:, g * 3072:(g + 1) * 3072],
                    X2_d[:, g * 3072:(g + 1) * 3072])
            for g in range(6):
                nc.sync.dma_start(
                    W2R_s[:, g * 12 * NJO:(g + 1) * 12 * NJO],
                    W2R_d[:, g * 12 * NJO:(g + 1) * 12 * NJO])
            nc.sync.dma_start(ONESB_s[:], ONESB_d[:])
            nc.sync.dma_start(RONESB_s[:], RONESB_d[:])

            tensors = (XT_s, X2_s, W2R_s, ONESB_s, RONESB_s)
            pools = (sp, vq, wc_pool, p_pool, dram_pool, ps_s, ps_m, ps_z)

            # modulo software pipeline: rep r's stage s lands in slot r+s
            stage_lists = [None] * reps
            for slot in range(reps + NST - 1):
                for s in range(NST - 1, -1, -1):
                    r = slot - s
                    if 0 <= r < reps:
                        if stage_lists[r] is None:
                            stage_lists[r] = make_stages(
                                nc, tc, tensors, pools, out_d, flags)
                        stage_lists[r][s]()

    nc.compile()
    return nc


def make_host_inputs(x, W):
    """Build per-core in_maps from the full inputs (r-major fp16 layouts)."""
    x = np.ascontiguousarray(np.asarray(x, dtype=np.float32))
    W = np.asarray(W, dtype=np.float32)
    f16 = np.float16
    # W2R[p, rb, i, j, o] = W[rb*128+p, j, o, i]
    W2R = np.ascontiguousarray(
        W.reshape(NRB, 128, J, O, I).transpose(1, 0, 4, 2, 3)
        .reshape(128, KT * NJO)).astype(f16)
    ONESB = np.ones((128, 1), f16)
    RONESB = np.ones((1, BL), f16)

    in_maps = []
    for c in range(N_CORES):
        xc = x[c * BL:(c + 1) * BL]                      # [64, R, I]
        XT = np.ascontiguousarray(
            xc.transpose(1, 2, 0).reshape(NRB, 128, I, BL)
            .transpose(1, 0, 2, 3).reshape(128, KT * BL)).astype(f16)
        X2 = np.ascontiguousarray(
            xc.reshape(BL, NRB, 128, I).transpose(0, 1, 3, 2)
            .reshape(BL, RI)).astype(f16)
        in_maps.append({
            "XT": XT,
            "X2": X2,
            "W2R": W2R,
            "ONESB": ONESB,
            "RONESB": RONESB,
        })
    return in_maps


def assemble_output(results):
    return np.concatenate(
        [results[c]["out"].reshape(BL, J, O, 1) for c in range(N_CORES)],
        axis=0).astype(np.float32)


_NC_CACHE = {}


def kernel(x, W):
    if "nc" not in _NC_CACHE:
        _NC_CACHE["nc"] = build_nc(reps=1)
    nc = _NC_CACHE["nc"]
    in_maps = make_host_inputs(x, W)
    res = run_bass_kernel_spmd(nc, in_maps, list(range(N_CORES)))
    return assemble_output(res.results)


if __name__ == "__main__":
    import reference
    inputs = reference.setup_inputs()
    expected = np.asarray(reference.reference(**inputs))
    got = kernel(np.asarray(inputs["x"]), np.asarray(inputs["W"]))
    err = np.abs(got - expected).max()
    rel = err / np.abs(expected).max()
    print("abs err:", err, "scale-rel err:", rel)


# revision 17
# speedup vs baseline: 9.7918x; 1.1266x over previous
"""Trainium2 Bass kernel for CapsNet DigitCaps dynamic routing (nn_DigitCaps).

Reference computation:
    u_hat[b,r,j,o] = W[r,j,o,:] @ x[b,r,:]        B,R,J,O,I = 512,1152,10,16,8
    b_ij = 0; 3 routing iterations:
        c = softmax(b_ij, axis=0)                  # over routes r, per j
        s[b,j,o] = sum_r c[r,j] * u_hat[b,r,j,o]
        v = squash(s) = s*|s|/(1+s^2)              # elementwise
        b_ij += mean_b sum_o u_hat[b,r,j,o]*v[b,j,o]
    return v[..., None]

Kernel strategy (data-parallel over batch, 8 cores, 64 rows each; u_hat is
never materialized). All matmul/elementwise operands fp16 (PE: 1 cyc per
output row at any N; DVE 2x/4x fast modes need 2-byte packed SBUF operands),
fp32 PSUM accumulation. rel err ~5e-3 vs the fp32 reference.

r-major layout: partition p = r % 128, free blocks rb = r // 128 (9 blocks),
so b_ij, e=exp(b), and b_upd all live as [128, (rb j)] = [128, 90] with no
cross-partition shuffles:
    s    = X @ (e-scaled W2); e expanded over o once per iter (eo) so the
           wc broadcast-mult is innermost-packed; softmax 1/Z deferred to
           squash (it commutes through the matmul).
    M_k  = X_k^T @ V per (rb,i) chunk; 3 of 4 PSUM blocks are ACT-copied to
           fp16 SBUF so the W2R (*) M products run at the DVE fast rate.
    b_upd[:, rb*10:..] = tree-add over i + small XY-reduce over (i,o).
    b_upd is AllGather-summed across cores (~15us NRT collective); softmax
    runs redundantly on every core.

Each rep is split into 7 stages A..G (s0 / M0 / AR0 / s1 / M1 / AR1 / s2)
and emitted as a modulo software pipeline across reps (rep r's stage s in
slot r+s): per-engine instruction queues execute in emission order, so the
interleave is what lets other reps' compute fill each rep's two collective
windows.
"""
import os
import numpy as np
from contextlib import ExitStack

import concourse.bacc as bacc
import concourse.bass as bass
import concourse.tile as tile
from concourse import mybir
from concourse.bass_utils import run_bass_kernel_spmd

F32 = mybir.dt.float32
F16 = mybir.dt.float16

B, R, J, O, I = 512, 1152, 10, 16, 8
N_CORES = 8
BL = B // N_CORES          # 64 batch rows per core
RI = R * I                 # 9216
NJO = J * O                # 160
NRB = 9                    # r-blocks of 128
KT = RI // 128             # 72 contraction chunks (= NRB * I)
NUM_ITER = 3
MST = 256                  # mps chunk stride (PSUM bank alignment)
NST = 7                    # pipeline stages per rep


def make_stages(nc, tc, tensors, pools, out_d, flags):
    (XT_s, X2_s, W2R_s, ONESB_s, RONESB_s) = tensors
    (sp, vq, wc_pool, p_pool, dram_pool, ps_s, ps_m, ps_z) = pools
    st = {}

    def emit_post_ar(it):
        """exp + eo + Z chain preparing iteration `it` (reads st['b_state'])."""
        e_s = sp.tile([128, 90], F16, tag="e", name="e_s")
        nc.scalar.activation(e_s[:], st['b_state'][:],
                             mybir.ActivationFunctionType.Exp,
                             scale=1.0 / B)
        eo = sp.tile([128, 90 * O], F16, tag="eo", name="eo")
        i0 = e_s[:].rearrange("p (rb j a) -> p rb j a", j=J, a=1)
        eov = eo[:].rearrange("p (rb j o) -> p rb j o", j=J, o=O)
        i0b, _ = bass.broadcast_tensor_aps(i0, eov)
        nc.scalar.activation(eov, i0b, mybir.ActivationFunctionType.Copy)
        zmix = ps_z.tile([BL, 100], F32, tag="z")
        zps = zmix[0:1, 0:90]
        nc.tensor.matmul(zps, ONESB_s[:, 0:1], e_s[:], start=True, stop=True)
        zsum = sp.tile([1, 10], F32, tag="zsum")
        nc.vector.tensor_reduce(
            zsum[:], zps.rearrange("p (rb j) -> p j rb", j=J),
            axis=mybir.AxisListType.X, op=mybir.AluOpType.add)
        zinv = sp.tile([1, 10], F32, tag="zinv")
        nc.vector.reciprocal(zinv[:], zsum[:])
        zinv16 = sp.tile([1, 10], F16, tag="zinv16")
        nc.vector.tensor_copy(zinv16[:], zinv[:])
        zb_ps = zmix[0:BL, 90:100]
        nc.tensor.matmul(zb_ps, RONESB_s[:, 0:BL], zinv16[:],
                         start=True, stop=True)
        zinv_b = sp.tile([BL, 10], F32, tag="zinv_b")
        nc.vector.tensor_copy(zinv_b[:], zb_ps)
        st[f'eo{it}'] = eo
        st[f'zinv_b{it}'] = zinv_b

    def stage_s(it):
        def f():
            scaled = it > 0 and "skip_scale" not in flags
            s_ps = ps_s.tile([BL, NJO], F32, tag="s")
            for rb in range(NRB):
                if scaled:
                    eo = st[f'eo{it}']
                    wc = wc_pool.tile([128, I * NJO], F16, tag="wc")
                    in0 = W2R_s[:, rb * I * NJO:(rb + 1) * I * NJO] \
                        .rearrange("p (i j o) -> p i j o", j=J, o=O)
                    in1 = eo[:, rb * J * O:(rb + 1) * J * O] \
                        .rearrange("p (a j o) -> p a j o", a=1, o=O)
                    i0b, i1b = bass.broadcast_tensor_aps(in0, in1)
                    nc.vector.tensor_tensor(
                        wc[:].rearrange("p (i j o) -> p i j o", j=J, o=O),
                        i0b, i1b, op=mybir.AluOpType.mult)
                for i in range(I):
                    k = rb * I + i
                    rhs = (wc[:, i * NJO:(i + 1) * NJO] if scaled
                           else W2R_s[:, k * NJO:(k + 1) * NJO])
                    nc.tensor.matmul(s_ps[:], XT_s[:, k * BL:(k + 1) * BL],
                                     rhs, start=(k == 0), stop=(k == KT - 1))

            # squash (deferred softmax normalization when scaled), fp16
            # chain after a single PSUM read; kscl folded into s16
            s16 = sp.tile([BL, NJO], F16, tag="s16", name="s16")
            if scaled:
                zinv_b = st[f'zinv_b{it}']
                i0 = s_ps[:].rearrange("p (j o) -> p j o", o=O)
                i1 = zinv_b[:].rearrange("p (j o) -> p j o", o=1)
                i0b, i1b = bass.broadcast_tensor_aps(i0, i1)
                nc.vector.tensor_tensor(
                    s16[:].rearrange("p (j o) -> p j o", o=O), i0b, i1b,
                    op=mybir.AluOpType.mult)
            else:
                nc.scalar.activation(s16[:], s_ps[:],
                                     mybir.ActivationFunctionType.Copy,
                                     scale=1.0 / R)
            src = s16
            with nc.allow_low_precision(reason="fp16 squash; |s|<20"):
                sneg = sp.tile([BL, NJO], F16, tag="sneg")
                nc.vector.tensor_scalar_mul(sneg[:], src[:], -1.0)
                sabs = sp.tile([BL, NJO], F16, tag="sabs")
                nc.vector.tensor_tensor(sabs[:], src[:], sneg[:],
                                        op=mybir.AluOpType.max)
                den1 = sp.tile([BL, NJO], F16, tag="den1")
                nc.vector.scalar_tensor_tensor(den1[:], src[:], 1.0,
                                               src[:],
                                               op0=mybir.AluOpType.mult,
                                               op1=mybir.AluOpType.mult)
                nc.vector.tensor_scalar_add(den1[:], den1[:], 1.0)
                rec = sp.tile([BL, NJO], F16, tag="rec")
                nc.vector.reciprocal(rec[:], den1[:])
                num = sp.tile([BL, NJO], F16, tag="num")
                nc.vector.tensor_mul(num[:], src[:], sabs[:])
            if it == NUM_ITER - 1:
                vout = vq.tile([BL, NJO], F32, tag="vout")
                nc.vector.tensor_mul(vout[:], num[:], rec[:])
                nc.sync.dma_start(out_d[:], vout[:])
            else:
                vpad = vq.tile([BL, NJO], F16, tag="vpad")
                nc.vector.tensor_mul(vpad[:], num[:], rec[:])
                st[f'vpad{it}'] = vpad
        return f

    def stage_m(it):
        def f():
            if "skip_m" in flags:
                b_upd = sp.tile([128, 90], F16, tag="b_upd")
                nc.vector.memset(b_upd[:], 0.001)
                st[f'b_upd{it}'] = b_upd
                return
            vpad = st[f'vpad{it}']
            b_upd = sp.tile([128, 90], F16, tag="b_upd")
            for rb in range(NRB):
                prb = p_pool.tile([128, I * NJO], F16, tag="prb")
                for half in range(2):
                    # two 2-chunk PSUM tiles ACT-copied into one 4-chunk fp16
                    # buffer, then a single batched DVE product
                    mcp = p_pool.tile([128, 4 * NJO], F16, tag="mcp")
                    for quarter in range(2):
                        mps = ps_m.tile([128, 2 * MST], F32, tag="m")
                        for q in range(2):
                            k = rb * I + half * 4 + quarter * 2 + q
                            nc.tensor.matmul(mps[:, q * MST:q * MST + NJO],
                                             X2_s[:, k * 128:(k + 1) * 128],
                                             vpad[:], start=True, stop=True)
                        mview = mps[:].rearrange("p (c n) -> p c n",
                                                 n=MST)[:, :, 0:NJO]
                        nc.scalar.activation(
                            mcp[:, quarter * 2 * NJO:(quarter + 1) * 2 * NJO]
                            .rearrange("p (c n) -> p c n", n=NJO),
                            mview, mybir.ActivationFunctionType.Copy)
                    c0 = half * 4
                    nc.vector.tensor_tensor(
                        prb[:, c0 * NJO:(c0 + 4) * NJO],
                        W2R_s[:, (rb * I + c0) * NJO:(rb * I + c0 + 4) * NJO],
                        mcp[:], op=mybir.AluOpType.mult)
                t1 = p_pool.tile([128, 4 * NJO], F16, tag="t1", name="t1")
                nc.vector.tensor_add(t1[:], prb[:, 0:4 * NJO],
                                     prb[:, 4 * NJO:8 * NJO])
                t2 = p_pool.tile([128, 2 * NJO], F16, tag="t2", name="t2")
                nc.vector.tensor_add(t2[:], t1[:, 0:2 * NJO],
                                     t1[:, 2 * NJO:4 * NJO])
                with nc.allow_low_precision(reason="fp16 b_upd; 0.05% rel"):
                    nc.vector.tensor_reduce(
                        b_upd[:, rb * J:(rb + 1) * J],
                        t2[:].rearrange("p (i j o) -> p j i o", j=J, o=O),
                        axis=mybir.AxisListType.XY, op=mybir.AluOpType.add)
            st[f'b_upd{it}'] = b_upd
        return f

    def stage_ar(it):
        def f():
            b_upd = st[f'b_upd{it}']
            cc_in = dram_pool.tile([128, 90], F16, tag="cc_in")
            nc.sync.dma_start(cc_in[:], b_upd[:])
            if "skip_ar" in flags:
                cc_out = dram_pool.tile([128, 90], F16, tag="cc_out")
                nc.sync.dma_start(cc_out[:], cc_in[:])
                gath = sp.tile([128, 8 * 90], F16, tag="gath")
                for kk in range(8):
                    nc.sync.dma_start(gath[:, kk * 90:(kk + 1) * 90],
                                      cc_out[:])
            else:
                cc_out = dram_pool.tile([N_CORES * 128, 90], F16,
                                        tag="cc_outg")
                nc.gpsimd.collective_compute(
                    "AllGather", mybir.AluOpType.bypass,
                    replica_groups=[list(range(N_CORES))],
                    ins=[cc_in.opt()], outs=[cc_out.opt()])
                gath = sp.tile([128, 8 * 90], F16, tag="gath")
                nc.sync.dma_start(
                    gath[:].rearrange("p (k f) -> p k f", f=90),
                    cc_out[:].rearrange("(k p) f -> p k f", p=128))
            if it == 0:
                b_state = sp.tile([128, 90], F16, tag="bstate0")
                with nc.allow_low_precision(reason="fp16 b state"):
                    nc.vector.tensor_reduce(
                        b_state[:],
                        gath[:].rearrange("p (k f) -> p f k", f=90),
                        axis=mybir.AxisListType.X, op=mybir.AluOpType.add)
            else:
                upd_g = sp.tile([128, 90], F16, tag="upd_g")
                with nc.allow_low_precision(reason="fp16 b state"):
                    nc.vector.tensor_reduce(
                        upd_g[:],
                        gath[:].rearrange("p (k f) -> p f k", f=90),
                        axis=mybir.AxisListType.X, op=mybir.AluOpType.add)
                b_state = sp.tile([128, 90], F16, tag="bstate1b")
                nc.vector.tensor_add(b_state[:], st['b_state'][:], upd_g[:])
            st['b_state'] = b_state
            emit_post_ar(it + 1)
        return f

    return [stage_s(0), stage_m(0), stage_ar(0),
            stage_s(1), stage_m(1), stage_ar(1),
            stage_s(2)]


def build_nc(reps=1, flags=()):
    nc = bacc.Bacc("TRN2", target_bir_lowering=False, debug=False,
                   num_devices=N_CORES)
    XT_d = nc.dram_tensor("XT", [128, KT * BL], F16, kind="ExternalInput")
    X2_d = nc.dram_tensor("X2", [BL, RI], F16, kind="ExternalInput")
    W2R_d = nc.dram_tensor("W2R", [128, KT * NJO], F16, kind="ExternalInput")
    ONESB_d = nc.dram_tensor("ONESB", [128, 1], F16, kind="ExternalInput")
    RONESB_d = nc.dram_tensor("RONESB", [1, BL], F16, kind="ExternalInput")
    out_d = nc.dram_tensor("out", [BL, NJO], F32, kind="ExternalOutput")

    with tile.TileContext(nc) as tc:
        with ExitStack() as ctx:
            pers = ctx.enter_context(tc.tile_pool(name="pers", bufs=1))
            sp = ctx.enter_context(tc.tile_pool(name="sp", bufs=4))
            vq = ctx.enter_context(tc.tile_pool(name="vq", bufs=3))
            wc_pool = ctx.enter_context(tc.tile_pool(name="wcp", bufs=3))
            p_pool = ctx.enter_context(tc.tile_pool(name="pp", bufs=3))
            dram_pool = ctx.enter_context(
                tc.tile_pool(name="dram", bufs=3, space="DRAM"))
            ps_s = ctx.enter_context(tc.tile_pool(name="ps_s", bufs=3, space="PSUM"))
            ps_m = ctx.enter_context(tc.tile_pool(name="ps_m", bufs=3, space="PSUM"))
            ps_z = ctx.enter_context(tc.tile_pool(name="ps_z", bufs=2, space="PSUM"))

            XT_s = pers.tile([128, KT * BL], F16)
            X2_s = pers.tile([BL, RI], F16)
            W2R_s = pers.tile([128, KT * NJO], F16)
            ONESB_s = pers.tile([128, 1], F16)
            RONESB_s = pers.tile([1, BL], F16)

            for g in range(3):
                nc.sync.dma_start(
                    XT_s[:, g * 24 * BL:(g + 1) * 24 * BL],
                    XT_d[:, g * 24 * BL:(g + 1) * 24 * BL])
                nc.sync.dma_start(
                    X2_s[:, g * 3072:(g + 1) * 3072],
                    X2_d[:, g * 3072:(g + 1) * 3072])
            for g in range(6):
                nc.sync.dma_start(
                    W2R_s[:, g * 12 * NJO:(g + 1) * 12 * NJO],
                    W2R_d[:, g * 12 * NJO:(g + 1) * 12 * NJO])
            nc.sync.dma_start(ONESB_s[:], ONESB_d[:])
            nc.sync.dma_start(RONESB_s[:], RONESB_d[:])

            tensors = (XT_s, X2_s, W2R_s, ONESB_s, RONESB_s)
            pools = (sp, vq, wc_pool, p_pool, dram_pool, ps_s, ps_m, ps_z)

            # modulo software pipeline: rep r's stage s lands in slot r+s
            stage_lists = [None] * reps
            for slot in range(reps + NST - 1):
                for s in range(NST - 1, -1, -1):
                    r = slot - s
                    if 0 <= r < reps:
                        if stage_lists[r] is None:
                            stage_lists[r] = make_stages(
                                nc, tc, tensors, pools, out_d, flags)
                        stage_lists[r][s]()

    nc.compile()
    return nc


def make_host_inputs(x, W):
    """Build per-core in_maps from the full inputs (r-major fp16 layouts)."""
    x = np.ascontiguousarray(np.asarray(x, dtype=np.float32))
    W = np.asarray(W, dtype=np.float32)
    f16 = np.float16
    # W2R[p, rb, i, j, o] = W[rb*128+p, j, o, i]
    W2R = np.ascontiguousarray(
        W.reshape(NRB, 128, J, O, I).transpose(1, 0, 4, 2, 3)
        .reshape(128, KT * NJO)).astype(f16)
    ONESB = np.ones((128, 1), f16)
    RONESB = np.ones((1, BL), f16)

    in_maps = []
    for c in range(N_CORES):
        xc = x[c * BL:(c + 1) * BL]                      # [64, R, I]
        XT = np.ascontiguousarray(
            xc.transpose(1, 2, 0).reshape(NRB, 128, I, BL)
            .transpose(1, 0, 2, 3).reshape(128, KT * BL)).astype(f16)
        X2 = np.ascontiguousarray(
            xc.reshape(BL, NRB, 128, I).transpose(0, 1, 3, 2)
            .reshape(BL, RI)).astype(f16)
        in_maps.append({
            "XT": XT,
            "X2": X2,
            "W2R": W2R,
            "ONESB": ONESB,
            "RONESB": RONESB,
        })
    return in_maps


def assemble_output(results):
    return np.concatenate(
        [results[c]["out"].reshape(BL, J, O, 1) for c in range(N_CORES)],
        axis=0).astype(np.float32)


_NC_CACHE = {}


def kernel(x, W):
    if "nc" not in _NC_CACHE:
        _NC_CACHE["nc"] = build_nc(reps=1)
    nc = _NC_CACHE["nc"]
    in_maps = make_host_inputs(x, W)
    res = run_bass_kernel_spmd(nc, in_maps, list(range(N_CORES)))
    return assemble_output(res.results)


if __name__ == "__main__":
    import reference
    inputs = reference.setup_inputs()
    expected = np.asarray(reference.reference(**inputs))
    got = kernel(np.asarray(inputs["x"]), np.asarray(inputs["W"]))
    err = np.abs(got - expected).max()
    rel = err / np.abs(expected).max()
    print("abs err:", err, "scale-rel err:", rel)
